# revision 7
# baseline (speedup 1.0000x reference)
"""DBRX MoE experts kernel for Trainium2 (8 NeuronCores, tensor-parallel).

Strategy (tensor-parallel over the intermediate dim I):
  - Host: router (softmax top-2 + renorm), gather tokens per expert into one
    concatenated stream (CT = sum of padded per-expert counts), weight layout
    packing (transpose + bf16 cast).
  - Device (SPMD, identical token stream on every core, I/8 = 512 slice of
    w1/v1/w2 per core): for each expert segment X_e [C_e, D]:
        h = silu(X_e @ w1_k.T) * (X_e @ v1_k.T)   (I8 = 512 intermediate)
        y_partial = h @ w2_k.T                    ([C_e, D], bf16)
    bf16 matmuls, fp32 PSUM accumulation. Exactly load-balanced: every core
    executes the same instruction stream.
  - Host: all-reduce the 8 partial y streams, apply gates, scatter-add.

Device data layouts (per core k), partition-major so DMAs are [128, F] blocks:
  xt  [nD, 128, CT]     bf16: xt[d, p, t]      = x_all[t, 128 d + p]
  w1t [E, 4, 128, D]    bf16: w1t[e, it, p, 128 d + m] = w1[e][512 k + 128 it + m, 128 d + p]
  v1t same as w1t
  w2t [E, 16, 128, I8]  bf16: w2t[e, dt, p, 128 ic + m] = w2[e][128 dt + m, 512 k + 128 ic + p]
  y   [nD, 128, CT]     bf16: y[dt, p, t]      = y_partial[t, 128 dt + p]
"""

import numpy as np

import concourse.bass as bass
from concourse import bacc, mybir, tile
from concourse.bass_utils import run_bass_kernel_spmd

BF16 = mybir.dt.bfloat16
F32 = mybir.dt.float32
NP_BF16 = mybir.dt.np(BF16)

P = 128  # partitions
D, E, I = 2048, 8, 4096
NCORES = 8
I8 = I // NCORES  # per-core intermediate slice
NI8 = I8 // P  # 4
ND = D // P  # 16
NCHUNK = 512  # max moving free dim per matmul (one fp32 PSUM bank)


def balanced_chunks(C):
    """Split token dim C into near-equal chunks <= NCHUNK (multiples of 4),
    so no matmul has a tiny moving dim (keeps LDWEIGHTS hidden)."""
    n = -(-C // NCHUNK)
    out, o = [], 0
    for i in range(n):
        if i == n - 1:
            cs = C - o
        else:
            cs = (-(-(C - o) // (n - i)) + 3) // 4 * 4
        cs = min(cs, NCHUNK)
        out.append((o, cs))
        o += cs
    assert o == C and all(0 < c <= NCHUNK for _, c in out), (C, out)
    return out


def build_nc(cps, num_devices=NCORES, iters=1):
    """Build the SPMD device program. cps: per-expert padded token counts."""
    offs = [0]
    for c in cps:
        offs.append(offs[-1] + c)
    CT = offs[-1]

    nc = bacc.Bacc(
        "TRN2", target_bir_lowering=False, debug=False, num_devices=num_devices
    )
    xt_d = nc.dram_tensor("xt", [ND, P, CT], BF16, kind="ExternalInput").ap()
    w1_d = nc.dram_tensor("w1t", [E, NI8, P, D], BF16, kind="ExternalInput").ap()
    v1_d = nc.dram_tensor("v1t", [E, NI8, P, D], BF16, kind="ExternalInput").ap()
    w2_d = nc.dram_tensor("w2t", [E, ND, P, I8], BF16, kind="ExternalInput").ap()
    y_d = nc.dram_tensor("y", [ND, P, CT], BF16, kind="ExternalOutput").ap()

    with tile.TileContext(nc) as tc:
        with (
            tc.tile_pool(name="xin", bufs=2) as xin,
            tc.tile_pool(name="h2p", bufs=2) as h2p,
            tc.tile_pool(name="wload", bufs=10) as wload,
            tc.tile_pool(name="w2load", bufs=8) as w2load,
            tc.tile_pool(name="yout", bufs=4) as yout,
            tc.tile_pool(name="sgp", bufs=6) as sgp,
            tc.tile_pool(name="ps", bufs=8, space="PSUM") as ps,
        ):
            for _rep in range(iters):
                xes = [None] * E

                def load_x(e, queues):
                    xe = xin.tile([P, ND, cps[e]], BF16, tag="xe")
                    o = offs[e]
                    for d in range(ND):
                        queues[d % len(queues)].dma_start(
                            xe[:, d, :], xt_d[d, :, o : o + cps[e]]
                        )
                    xes[e] = xe

                def load_x0():
                    # startup: chunk-column-sliced load across two idle queues
                    # so the first matmul group can start after chunk 0 only
                    xe = xin.tile([P, ND, cps[0]], BF16, tag="xe")
                    qs = [nc.gpsimd, nc.scalar]
                    i = 0
                    for co, cs in balanced_chunks(cps[0]):
                        for d in range(ND):
                            qs[i % 2].dma_start(
                                xe[:, d, co : co + cs], xt_d[d, :, co : co + cs]
                            )
                            i += 1
                    xes[0] = xe

                load_x0()
                for e in range(E):
                    Cp = cps[e]
                    chunks = balanced_chunks(Cp)
                    xe = xes[e]
                    h2 = h2p.tile([P, NI8, Cp], BF16, tag="h2")

                    # Phase 1: h2 = silu(x@w1.T) * (x@v1.T), laid out [I8_part, C]
                    for it in range(NI8):
                        w1sb = wload.tile([P, ND, P], BF16, tag="w")
                        v1sb = wload.tile([P, ND, P], BF16, tag="w")
                        nc.sync.dma_start(w1sb[:], w1_d[e, it])
                        nc.sync.dma_start(v1sb[:], v1_d[e, it])
                        for co, cs in chunks:
                            ph = ps.tile([P, NCHUNK], F32, tag="pp")
                            pg = ps.tile([P, NCHUNK], F32, tag="pp")
                            for d in range(ND):
                                nc.tensor.matmul(
                                    ph[:, :cs],
                                    w1sb[:, d, :],
                                    xe[:, d, co : co + cs],
                                    start=(d == 0),
                                    stop=(d == ND - 1),
                                )
                            for d in range(ND):
                                nc.tensor.matmul(
                                    pg[:, :cs],
                                    v1sb[:, d, :],
                                    xe[:, d, co : co + cs],
                                    start=(d == 0),
                                    stop=(d == ND - 1),
                                )
                            s = sgp.tile([P, NCHUNK], F32, tag="sg")
                            nc.scalar.activation(
                                s[:, :cs],
                                ph[:, :cs],
                                mybir.ActivationFunctionType.Silu,
                            )
                            nc.vector.tensor_mul(
                                h2[:, it, co : co + cs], s[:, :cs], pg[:, :cs]
                            )

                    # prefetch next expert's tokens while phase 2 runs
                    # (scalar queue is still idle before the first y stores)
                    if e + 1 < E:
                        load_x(e + 1, [nc.gpsimd, nc.scalar] if e == 0 else [nc.gpsimd])

                    # Phase 2: y_partial = h2.T @ w2.T, laid out [D_part, C]
                    o = offs[e]
                    for dt in range(ND):
                        w2sb = w2load.tile([P, NI8, P], BF16, tag="w2")
                        nc.gpsimd.dma_start(w2sb[:], w2_d[e, dt])
                        ysb = yout.tile([P, Cp], BF16, tag="y")
                        for co, cs in chunks:
                            py = ps.tile([P, NCHUNK], F32, tag="pp")
                            for ic in range(NI8):
                                nc.tensor.matmul(
                                    py[:, :cs],
                                    w2sb[:, ic, :],
                                    h2[:, ic, co : co + cs],
                                    start=(ic == 0),
                                    stop=(ic == NI8 - 1),
                                )
                            nc.vector.tensor_copy(ysb[:, co : co + cs], py[:, :cs])
                        nc.scalar.dma_start(y_d[dt, :, o : o + Cp], ysb[:])

    nc.compile()
    return nc


def pack_x(x_all):
    """[CT, D] f32 -> [nD, 128, CT] bf16."""
    CT = x_all.shape[0]
    return np.ascontiguousarray(x_all.T.reshape(ND, P, CT)).astype(NP_BF16)


def pack_w_up(w, k):
    """w1/v1 [E, I, D] -> core k's [E, NI8, 128, D] bf16 lhsT tiles."""
    a = w[:, k * I8 : (k + 1) * I8, :]  # [E, I8, D]
    a = a.reshape(E, NI8, P, ND, P)  # [e, it, m, d, p]
    a = a.transpose(0, 1, 4, 3, 2)  # [e, it, p, d, m]
    return np.ascontiguousarray(a.reshape(E, NI8, P, D)).astype(NP_BF16)


def pack_w_down(w, k):
    """w2 [E, D, I] -> core k's [E, ND, 128, I8] bf16 lhsT tiles."""
    a = w[:, :, k * I8 : (k + 1) * I8]  # [E, D, I8]
    a = a.reshape(E, ND, P, NI8, P)  # [e, dt, m, ic, p]
    a = a.transpose(0, 1, 4, 3, 2)  # [e, dt, p, ic, m]
    return np.ascontiguousarray(a.reshape(E, ND, P, I8)).astype(NP_BF16)


def route(x, wr, top_k=2):
    """Softmax top-k with renormalization. Returns topi [T,k], topw [T,k]."""
    logits = x @ wr.T
    logits -= logits.max(-1, keepdims=True)
    p = np.exp(logits, dtype=np.float32)
    p /= p.sum(-1, keepdims=True)
    topi = np.argpartition(-p, top_k - 1, axis=-1)[:, :top_k]
    topw = np.take_along_axis(p, topi, -1)
    topw = topw / topw.sum(-1, keepdims=True)
    return topi, topw


def plan(x, wr):
    """Routing plan: per-expert indices, gates, padded counts, offsets."""
    topi, topw = route(x, wr)
    idx = [np.nonzero((topi == e).any(-1))[0] for e in range(E)]
    gates = np.zeros((x.shape[0], E), np.float32)
    np.put_along_axis(gates, topi, topw, axis=-1)
    cps = tuple(max(64, ((len(ix) + 7) // 8) * 8) for ix in idx)
    offs = [0]
    for c in cps:
        offs.append(offs[-1] + c)
    return idx, gates, cps, offs


def make_in_maps(x, w1, v1, w2, idx, cps, offs):
    CT = offs[-1]
    x_all = np.zeros((CT, D), np.float32)
    for e in range(E):
        x_all[offs[e] : offs[e] + len(idx[e])] = x[idx[e]]
    xt = pack_x(x_all)
    return [
        {
            "xt": xt,
            "w1t": pack_w_up(w1, k),
            "v1t": pack_w_up(v1, k),
            "w2t": pack_w_down(w2, k),
        }
        for k in range(NCORES)
    ]


_NC_CACHE = {}


def kernel(hidden_states, wr, w1, v1, w2, index):
    x = np.asarray(hidden_states, dtype=np.float32)
    wr = np.asarray(wr, dtype=np.float32)
    w1 = np.asarray(w1, dtype=np.float32)
    v1 = np.asarray(v1, dtype=np.float32)
    w2 = np.asarray(w2, dtype=np.float32)
    T = x.shape[0]

    idx, gates, cps, offs = plan(x, wr)

    if cps not in _NC_CACHE:
        _NC_CACHE[cps] = build_nc(cps)
    nc = _NC_CACHE[cps]

    in_maps = make_in_maps(x, w1, v1, w2, idx, cps, offs)
    res = run_bass_kernel_spmd(nc, in_maps, core_ids=list(range(NCORES)))

    ysum = np.zeros((ND, P, offs[-1]), np.float32)
    for k in range(NCORES):
        ysum += res.results[k]["y"].astype(np.float32)
    y_all = ysum.transpose(2, 0, 1).reshape(offs[-1], D)

    out = np.zeros((T, D), np.float32)
    for e in range(E):
        ix = idx[e]
        out[ix] += gates[ix, e][:, None] * y_all[offs[e] : offs[e] + len(ix)]
    return out
